# revision 40
# baseline (speedup 1.0000x reference)
"""Trainium2 Bass kernel for EnhancedTransformerBlock (B=2,T=2048,C=1024,H=16,
SwiGLU HIDDEN=2730, ALiBi-abs + causal attention).

Sharding over 8 cores: batch (2) x head-groups (4 heads/core). Heads are
assigned to cores in decay-sorted slots (slot s on core g = global head
[12,8,4,0][s]+g) so that the per-slot ALiBi key-tile cutoffs [16,15,4,1]
are identical across cores; key tiles whose exp(-slope*k) factor underflows
are skipped structurally (the reference ADDS slope*|q-k|, so weights decay
as exp(-slope*k) after causal masking). LN1 is folded into the QKV matmuls
via a mean-row augmentation plus a per-token rstd scale on the PSUM->SBUF
copy. Attention runs with transposed scores S^T[tk,tq]; the ALiBi key-side
factor is folded into V's rows, the softmax denominator rides as a ck
column through the PV matmul, and causally-dead query columns are trimmed
from diagonal tiles. Attention outputs O^T are exchanged per slot with a
small AllToAll (4x256KB instead of reduce-scattering 4x1MB proj partials);
the full projection then runs on the destination core, accumulating into
an fp32 row accumulator initialized with the residual. The SwiGLU MLP runs
row-parallel with fp8(e4m3) DoubleRow matmuls on contiguous 512-row blocks.
"""
import sys, types
sys.path.insert(0, "/opt/trn_rl_repo")
import numpy as np
import ml_dtypes

import concourse.bass as bass
import concourse.tile as tile
from concourse import mybir
import concourse.bass_utils as bass_utils
import bass_rust

# ----------------------------------------------------------------------------
# environment patches (walrus in this container accepts only 1 sync-wait/inst)
# ----------------------------------------------------------------------------
_DRAIN_WAIT_LIMIT = 1

def _patched_drain_and_barrier(self, tick_clock, wait_clock):
    nc = self.nc
    drain_inst = nc.sync.drain()
    wait_clock.add_sem_waits(
        drain_inst.ins, bass_rust.ScopedClock({None: tick_clock.global_clock})
    )
    si = drain_inst.ins.sync_info
    waits = list(si.on_wait) if si is not None else []
    if len(waits) > _DRAIN_WAIT_LIMIT:
        si.on_wait = waits[:_DRAIN_WAIT_LIMIT]
        for i in range(_DRAIN_WAIT_LIMIT, len(waits), _DRAIN_WAIT_LIMIT):
            d2 = nc.sync.drain()
            d2.ins.sync_info = bass_rust.SyncInfo(
                on_wait=waits[i:i + _DRAIN_WAIT_LIMIT], on_update=[]
            )
    nc.all_engine_barrier()
    popped = nc._tile_sem_poison_stack.pop()
    assert popped is self._sem_poison
    nc.clear_and_free_semaphores(list(self.sems.allocated().values()))
    nc.all_engine_barrier()


def _split_excess_waits(nc, limit=_DRAIN_WAIT_LIMIT):
    n = [0]
    for bb in nc.main_func.blocks:
        insts = bb.instructions
        out = []
        changed = False
        for inst in insts:
            si = inst.sync_info
            waits = list(si.on_wait) if si is not None else []
            if len(waits) > limit:
                changed = True
                keep = waits[-limit:]
                rest = waits[:-limit]
                for i in range(0, len(rest), limit):
                    n[0] += 1
                    d = mybir.InstNoOp(
                        name=f"waitsplit-{n[0]}", engine=inst.engine, ins=[], outs=[]
                    )
                    d.sync_info = bass_rust.SyncInfo(
                        on_wait=rest[i:i + limit], on_update=[]
                    )
                    out.append(d)
                si.on_wait = keep
            out.append(inst)
        if changed:
            bb.instructions = out


def _install_patches():
    tile.TileContext._drain_and_barrier = _patched_drain_and_barrier
    if "antenv.axon_hooks" not in sys.modules:
        try:
            from trn_agent_boot.trn_boot import _ntff_profile_via_ctypes
            hook = _ntff_profile_via_ctypes("/opt/axon/libaxon_pjrt.so")
        except Exception:
            hook = None
        mod = types.ModuleType("antenv.axon_hooks")
        mod.get_axon_ntff_profile_hook = lambda: hook
        mod.set_axon_ntff_profile_hook = lambda h: None
        sys.modules["antenv.axon_hooks"] = mod
        bass_utils.upload_artifacts = lambda tmpdir: tmpdir

_install_patches()

# ----------------------------------------------------------------------------
# problem constants
# ----------------------------------------------------------------------------
B, T, C = 2, 2048, 1024
H, D = 16, 64
HID, HIDP = 2730, 2816
N_CORES, TP = 8, 4
ROWS = T // TP          # 512 contiguous rows per core after the exchange
HL = 4                  # local heads per core (one per slot)
EPS = 1e-5
CBIAS = 24.0
F32, BF16 = mybir.dt.float32, mybir.dt.bfloat16
F8 = mybir.dt.float8e4
bf16 = ml_dtypes.bfloat16
f8np = ml_dtypes.float8_e4m3
ts = bass.ts

NTT = T // 128          # 16 token tiles
NTB = T // 512          # 4 token blocks
CCH = C // 128          # 8 feature chunks
MCH = HIDP // 128       # 22 hidden chunks
RT = ROWS // 128        # 4 row tiles per core
GROUPS = [[0, 1, 2, 3], [4, 5, 6, 7]]
DR = mybir.MatmulPerfMode.DoubleRow

# slot s on core g holds global head SLOT_BASE[s] + g (0-indexed heads).
SLOT_BASE = [12, 8, 4, 0]
# per-slot key-tile cutoff: tile t is kept iff t < TCAP[s]. Computed from
# slope*128*t >= 30 (dropped relative weight < e^{-30+2*Smax} ~ 1e-8).
TCAP = [16, 15, 4, 1]


def _slopes():
    i = np.arange(1, H + 1, dtype=np.float64)
    return 1.0 / np.power(2.0, 8.0 * i / H)


# ----------------------------------------------------------------------------
# device program (identical on all 8 cores; per-core behavior via input data)
# ----------------------------------------------------------------------------
def _build(has_b1):
    K9 = 2 if has_b1 else 1
    nc = bass.Bass("TRN2", num_devices=N_CORES)

    xT_d = nc.dram_tensor("xT", [128, CCH, T], BF16, kind="ExternalInput")
    xtok_d = nc.dram_tensor("xtok", [T, C], BF16, kind="ExternalInput")
    xr_d = nc.dram_tensor("x_rows", [ROWS, C], F32, kind="ExternalInput")
    wq_d = nc.dram_tensor("wq", [128, 2, CCH, 128], BF16, kind="ExternalInput")
    wk_d = nc.dram_tensor("wk", [128, 2, CCH, 128], BF16, kind="ExternalInput")
    wv_d = nc.dram_tensor("wv", [128, CCH, 256], BF16, kind="ExternalInput")
    wq9_d = nc.dram_tensor("wq9", [2, 2, 128], BF16, kind="ExternalInput")
    wk9_d = nc.dram_tensor("wk9", [2, 2, 128], BF16, kind="ExternalInput")
    wv9_d = nc.dram_tensor("wv9", [2, 256], BF16, kind="ExternalInput")
    wp_d = nc.dram_tensor("wproj", [128, CCH, 1024], BF16, kind="ExternalInput")
    pm_d = nc.dram_tensor("pmask", [128, 4], F32, kind="ExternalInput")
    wg_d = nc.dram_tensor("wg", [128, MCH, CCH, 128], F8, kind="ExternalInput")
    wu_d = nc.dram_tensor("wu", [128, MCH, CCH, 128], F8, kind="ExternalInput")
    wd_d = nc.dram_tensor("wd", [128, MCH, 1024], F8, kind="ExternalInput")
    bg_d = nc.dram_tensor("bg", [128, MCH], F32, kind="ExternalInput")
    bu_d = nc.dram_tensor("bu", [128, MCH], F32, kind="ExternalInput")
    db_d = nc.dram_tensor("dbias", [128, 1024], F32, kind="ExternalInput")
    mk_d = nc.dram_tensor("masks", [128, 512], BF16, kind="ExternalInput")
    ckb_d = nc.dram_tensor("ckb", [128, NTT, HL], BF16, kind="ExternalInput")
    ckf_d = nc.dram_tensor("ckf", [128, NTT, HL], F32, kind="ExternalInput")
    sel2_d = nc.dram_tensor("sel2", [128, 128], BF16, kind="ExternalInput")

    out_d = nc.dram_tensor("out_rows", [ROWS, C], F32, kind="ExternalOutput")

    from contextlib import ExitStack
    with tile.TileContext(nc) as tc, ExitStack() as top:
        consts = top.enter_context(tc.tile_pool(name="consts", bufs=1))
        stats = top.enter_context(tc.tile_pool(name="stats", bufs=4))
        work = top.enter_context(tc.tile_pool(name="work", bufs=3))
        dramp = top.enter_context(tc.tile_pool(name="dram", bufs=1, space="DRAM"))

        eps_ap = consts.tile([128, 1], F32)
        nc.vector.memset(eps_ap[:], EPS)
        cb24 = consts.tile([128, 1], F32)
        nc.vector.memset(cb24[:], -CBIAS)
        ones1 = consts.tile([1, 128], F32)
        nc.vector.memset(ones1[:], 1.0)
        ck32 = consts.tile([128, NTT, HL], F32)
        # tiles whose DMAs are deferred past the critical QKV weights
        bg_sb = consts.tile([128, MCH], F32)
        bu_sb = consts.tile([128, MCH], F32)
        ident = consts.tile([128, 128], BF16)
        pmsk = consts.tile([128, 4], F32)
        wd_sb = consts.tile([128, MCH, 1024], F8)
        db_sb = consts.tile([128, 1024], F32)

        # persistent cross-phase state
        pacc = consts.tile([128, RT, C], F32)     # residual + proj accumulator
        y2T8 = consts.tile([128, CCH, ROWS], F8)
        mv2 = [consts.tile([128, 2], F32, name=f"mv2_{r}") for r in range(RT)]

        at_in = [dramp.tile([2 * NTB, 64, 512], F8, name=f"atin{s}")
                 for s in range(4)]
        dz_in = dramp.tile([8, 128], F8, name="dzin")
        dz_out = dramp.tile([8, 128], F8, name="dzout")
        at_out = [dramp.tile([2 * NTB, 64, 512], F8, name=f"atout{s}")
                  for s in range(4)]

        with ExitStack() as attn_scope:
            apool = attn_scope.enter_context(tc.tile_pool(name="attn", bufs=1))
            pipe = attn_scope.enter_context(tc.tile_pool(name="pipe", bufs=4))
            OnTp = attn_scope.enter_context(tc.tile_pool(name="OnTp", bufs=3))
            recp = attn_scope.enter_context(tc.tile_pool(name="recp", bufs=2))
            psA = attn_scope.enter_context(
                tc.tile_pool(name="psA", bufs=2, space="PSUM"))
            psS = attn_scope.enter_context(
                tc.tile_pool(name="psS", bufs=3, space="PSUM"))
            psO = attn_scope.enter_context(
                tc.tile_pool(name="psO", bufs=3, space="PSUM"))

            # qT/kT: [d 0:64 = slot 2p head, 64:128 = slot 2p+1 head][pair][T]
            qT = apool.tile([128, 2, T], BF16)
            kT = apool.tile([128, 2, T], BF16)
            # critical-path weight DMAs first on the gpsimd queue
            zt8 = apool.tile([1, 8, 128], F8)
            nc.vector.memset(zt8[:], 0.0)
            nc.sync.dma_start(dz_in[:, :], zt8[0, :, :])
            nc.gpsimd.collective_compute(
                "AllToAll", mybir.AluOpType.bypass,
                replica_groups=[[0, 1, 2, 3, 4, 5, 6, 7]],
                ins=[dz_in.opt()], outs=[dz_out.opt()],
            )
            wq_sb = apool.tile([128, 2, CCH, 128], BF16)
            nc.gpsimd.dma_start(wq_sb[:], wq_d[:, :, :, :])
            wk_sb = apool.tile([128, 2, CCH, 128], BF16)
            nc.gpsimd.dma_start(wk_sb[:], wk_d[:, :, :, :])
            wv_sb = apool.tile([128, CCH, 256], BF16)
            nc.gpsimd.dma_start(wv_sb[:], wv_d[:, :, :])
            wq9_sb = apool.tile([2, 2, 128], BF16)
            nc.gpsimd.dma_start(wq9_sb[:], wq9_d[:, :, :])
            wk9_sb = apool.tile([2, 2, 128], BF16)
            nc.gpsimd.dma_start(wk9_sb[:], wk9_d[:, :, :])
            wv9_sb = apool.tile([2, 256], BF16)
            nc.gpsimd.dma_start(wv9_sb[:], wv9_d[:, :])
            tri = apool.tile([128, 512], BF16)
            nc.gpsimd.dma_start(tri[:], mk_d[:, :])
            nc.gpsimd.dma_start(ck32[:], ckf_d[:, :, :])
            sel2 = apool.tile([128, 128], BF16)
            nc.gpsimd.dma_start(sel2[:], sel2_d[:, :])
            dpad2s = []
            for i in range(2):
                dp = apool.tile([128, 512], BF16, name=f"dpad{i}")
                nc.vector.memset(dp[:], 0.0)
                dpad2s.append(dp)
            Vh = apool.tile([128, NTT, HL, 66], BF16)
            nc.vector.memset(Vh[:, :, :, 65:66], 0.0)
            nc.gpsimd.dma_start(Vh[:, :, :, 64:65], ckb_d[:, :, :])
            wp_sb = apool.tile([128, CCH, 1024], BF16)
            for rt in range(RT):
                nc.scalar.dma_start(pacc[:, rt, :], xr_d[ts(rt, 128), :])

            with ExitStack() as qkv_scope:
                qpool = qkv_scope.enter_context(tc.tile_pool(name="qkv", bufs=1))

                xT_sb = qpool.tile([128, CCH, T], BF16)
                for cc in range(CCH):
                    nc.sync.dma_start(xT_sb[:, cc, :], xT_d[:, cc, :])

                mrow = qpool.tile([2, T], BF16)    # row0: -mu, row1: ones(b1)
                nc.vector.memset(mrow[:], 0.0)
                if has_b1:
                    nc.vector.memset(mrow[1:2, :], 1.0)
                rstd_row = qpool.tile([1, T], F32)
                rstdb = [qpool.tile([128, 512], F32, name=f"rstdb{b}")
                         for b in range(NTB)]
                rstds = [qpool.tile([128, 1], F32, name=f"rstds{t}")
                         for t in range(NTT)]

                def emit_stats(b):
                    # per-token-tile LN stats for block b (vector engine)
                    mucB = stats.tile([128, 32], F32, tag="mucB", bufs=2)
                    nc.vector.memset(mucB[:], 0.0)
                    for i in range(4):
                        t = 4 * b + i
                        xt = work.tile([128, C], BF16, tag="xt", bufs=2)
                        nc.scalar.dma_start(xt[:], xtok_d[ts(t, 128), :])
                        bst = stats.tile([128, 2, 6], F32, tag="bst")
                        for sg in range(2):
                            nc.vector.bn_stats(bst[:, sg, :], xt[:, ts(sg, 512)])
                        mv = stats.tile([128, 2], F32, tag="mv")
                        nc.vector.bn_aggr(mv[:], bst[:])
                        std = stats.tile([128, 1], F32, tag="std")
                        nc.scalar.activation(std[:], mv[:, 1:2],
                                             mybir.ActivationFunctionType.Sqrt,
                                             bias=eps_ap[:])
                        nc.vector.reciprocal(rstds[t][:], std[:])
                        nc.vector.tensor_scalar_mul(mucB[:, i:i + 1],
                                                    mv[:, 0:1], -1.0)
                        nc.vector.tensor_copy(mucB[:, 4 + i:5 + i], rstds[t][:])
                    stt = stats.tile([128, 32], F32, tag="stt")
                    nc.vector.transpose(stt[:], mucB[:])
                    sTb = stats.tile([8, 128], BF16, tag="sTb")
                    sTf = stats.tile([8, 128], F32, tag="sTf")
                    for i in range(4):
                        nc.vector.tensor_copy(sTb[0:8, ts(i, 32)],
                                              stt[32 * i:32 * i + 8, :])
                        nc.vector.tensor_copy(sTf[0:8, ts(i, 32)],
                                              stt[32 * i:32 * i + 8, :])
                    for i in range(4):
                        nc.scalar.dma_start(
                            mrow[0:1, 512 * b + 128 * i:512 * b + 128 * (i + 1)],
                            sTb[i:i + 1, :])
                        nc.scalar.dma_start(
                            rstd_row[0:1, 512 * b + 128 * i:512 * b + 128 * (i + 1)],
                            sTf[4 + i:5 + i, :])

                def emit_qkv(b):
                    def qk_group(p, wi, wsb, w9):
                        ps = psA.tile([128, 512], F32, tag="ps",
                                      name=f"qk_{b}_{p}_{wi}")
                        for cc in range(CCH):
                            nc.tensor.matmul(
                                ps[:], wsb[:, p, cc, :],
                                xT_sb[:, cc, ts(b, 512)],
                                start=(cc == 0), stop=False)
                        nc.tensor.matmul(
                            ps[:], w9[0:K9, p, :],
                            mrow[0:K9, ts(b, 512)],
                            start=False, stop=True)
                        return ps

                    ps_q0 = qk_group(0, 0, wq_sb, wq9_sb)
                    ps_k0 = qk_group(0, 1, wk_sb, wk9_sb)
                    # rstd broadcast for the block
                    psb = psS.tile([128, 512], F32, tag="s", name=f"psb_{b}")
                    nc.tensor.matmul(psb[:], ones1[0:1, :],
                                     rstd_row[0:1, ts(b, 512)],
                                     start=True, stop=True)
                    nc.scalar.copy(rstdb[b][:], psb[:])
                    nc.vector.tensor_tensor(qT[:, 0, ts(b, 512)], ps_q0[:],
                                            rstdb[b][:], mybir.AluOpType.mult)
                    nc.vector.tensor_tensor(kT[:, 0, ts(b, 512)], ps_k0[:],
                                            rstdb[b][:], mybir.AluOpType.mult)
                    ps_q1 = qk_group(1, 0, wq_sb, wq9_sb)
                    ps_k1 = qk_group(1, 1, wk_sb, wk9_sb)
                    nc.vector.tensor_tensor(qT[:, 1, ts(b, 512)], ps_q1[:],
                                            rstdb[b][:], mybir.AluOpType.mult)
                    nc.vector.tensor_tensor(kT[:, 1, ts(b, 512)], ps_k1[:],
                                            rstdb[b][:], mybir.AluOpType.mult)
                    for i in range(4):
                        t = 4 * b + i
                        psv = psA.tile([128, 256], F32, tag="ps", name=f"v_{t}")
                        for cc in range(CCH):
                            nc.tensor.matmul(psv[:], xT_sb[:, cc, ts(t, 128)],
                                             wv_sb[:, cc, :],
                                             start=(cc == 0), stop=False)
                        nc.tensor.matmul(psv[:], mrow[0:K9, ts(t, 128)],
                                         wv9_sb[0:K9, :], start=False, stop=True)
                        for h in range(HL):
                            nc.vector.tensor_scalar(
                                Vh[:, t, h, 0:64], psv[:, ts(h, 64)],
                                rstds[t][:], ck32[:, t, h:h + 1],
                                mybir.AluOpType.mult, mybir.AluOpType.mult)

                # ---- attention for pair p, query block c -----------------
                npair = [0]

                def emit_attn(p, c):
                    caps = (min(TCAP[2 * p], 4 * c + 4),
                            min(TCAP[2 * p + 1], 4 * c + 4))
                    t_hi = max(caps)
                    pos = [psO.tile([66, 512], F32, tag="po",
                                    name=f"po{j}_{c}_{p}") for j in range(2)]

                    def emit_s(t):
                        # S^T tiles for both heads of the pair at key tile t
                        if t >= 4 * c:
                            w = 512 - 128 * (t - 4 * c)
                            diag = True
                        else:
                            w, diag = 512, False
                        q0 = 512 * c + 512 - w
                        outs = {}
                        for j in range(2):
                            if t >= caps[j]:
                                continue
                            r0 = 64 * j
                            st = psS.tile([128, 512], F32, tag="s",
                                          name=f"s_{c}_{p}_{j}_{t}")
                            nc.tensor.matmul(st[:, 0:w],
                                             kT[r0:r0 + 64, p, ts(t, 128)],
                                             qT[r0:r0 + 64, p, q0:q0 + w],
                                             start=True, stop=True)
                            pT = pipe.tile([128, 512], BF16, tag="pT",
                                           name=f"pT_{c}_{p}_{j}_{t}")
                            nc.scalar.activation(
                                pT[:, 0:w], st[:, 0:w],
                                mybir.ActivationFunctionType.Exp,
                                bias=cb24[:])
                            if diag:
                                nc.vector.tensor_tensor(
                                    pT[:, 0:w], pT[:, 0:w],
                                    tri[:, 0:w], mybir.AluOpType.mult)
                            outs[j] = (pT, w)
                        return outs

                    pTs = {0: emit_s(0)}
                    if t_hi > 1:
                        pTs[1] = emit_s(1)
                    for t in range(t_hi):
                        outs = pTs.pop(t)
                        for j in range(2):
                            if j not in outs:
                                continue
                            pT, w = outs[j]
                            nc.tensor.matmul(pos[j][:, 512 - w:512],
                                             Vh[:, t, 2 * p + j, 0:66],
                                             pT[:, 0:w], start=(t == 0),
                                             stop=(t == caps[j] - 1),
                                             skip_group_check=True)
                        if t + 2 < t_hi:
                            pTs[t + 2] = emit_s(t + 2)

                    return pos

                def emit_den(p, c, pos):
                    # softmax denominators for the pair -> broadcast
                    dp = dpad2s[npair[0] % 2]
                    npair[0] += 1
                    nc.vector.tensor_copy(dp[0:1, :], pos[0][64:65, :])
                    nc.vector.tensor_copy(dp[64:65, :], pos[1][64:65, :])
                    rb = psS.tile([128, 512], F32, tag="s", name=f"rb_{c}_{p}")
                    nc.tensor.matmul(rb[:], sel2[:], dp[:],
                                     start=True, stop=True)
                    rec = recp.tile([128, 512], F32, tag="rec")
                    nc.vector.reciprocal(rec[:], rb[:])
                    OnT_c = OnTp.tile([128, 512], F8, tag="OnT")
                    nc.vector.tensor_tensor(OnT_c[0:64, :],
                                            pos[0][0:64, :], rec[0:64, :],
                                            mybir.AluOpType.mult)
                    nc.vector.tensor_tensor(OnT_c[64:128, :],
                                            pos[1][0:64, :],
                                            rec[64:128, :],
                                            mybir.AluOpType.mult)
                    for j in range(2):
                        # write to both batches' destination slots (the
                        # other batch's copy is zero-masked before proj)
                        nc.sync.dma_start(at_in[2 * p + j][c, :, :],
                                          OnT_c[64 * j:64 * j + 64, :])
                        nc.sync.dma_start(at_in[2 * p + j][4 + c, :, :],
                                          OnT_c[64 * j:64 * j + 64, :])

                # ---- emission: stats ahead of QKV, slot 0 after ----------
                emit_stats(0)
                emit_stats(1)
                emit_stats(2)
                emit_stats(3)
                emit_qkv(0)
                emit_qkv(1)
                emit_qkv(2)
                emit_qkv(3)
                for c in range(NTB):
                    emit_den(0, c, emit_attn(0, c))

            ALL8 = [[0, 1, 2, 3, 4, 5, 6, 7]]

            def emit_at(s):
                nc.gpsimd.collective_compute(
                    "AllToAll", mybir.AluOpType.bypass,
                    replica_groups=ALL8,
                    ins=[at_in[s].opt()], outs=[at_out[s].opt()],
                )

            def emit_proj(s):
                ot8 = OnTp.tile([128, 4, 512], F8, tag="ot8", bufs=2,
                                name=f"ot8_{s}")
                ot = OnTp.tile([128, 4, 512], BF16, tag="ot", bufs=2,
                               name=f"ot_{s}")
                for u in range(4):
                    nc.sync.dma_start(ot8[:, u, :],
                                      at_out[s][2 * u:2 * u + 2, :, :])
                    # upcast to bf16, zeroing the other batch's junk chunk
                    nc.vector.tensor_scalar_mul(ot[:, u, :], ot8[:, u, :],
                                                pmsk[:, u:u + 1])
                for rt in range(RT):
                    for nb in range(2):
                        pp = psA.tile([128, 512], F32, tag="ps",
                                      name=f"pj_{s}_{rt}_{nb}")
                        for u in range(4):
                            nc.tensor.matmul(
                                pp[:], ot[:, u, ts(rt, 128)],
                                wp_sb[:, 2 * s + (u % 2), ts(nb, 512)],
                                start=(u == 0), stop=(u == 3))
                        nc.vector.tensor_tensor(pacc[:, rt, ts(nb, 512)],
                                                pacc[:, rt, ts(nb, 512)],
                                                pp[:], mybir.AluOpType.add)

            emit_at(0)
            # deferred non-critical DMAs: dispatched once slot-0 work is off
            # the queues, landing during the attention phase
            nc.scalar.dma_start(wp_sb[:, 0:4, :], wp_d[:, 0:4, :])
            nc.scalar.dma_start(wp_sb[:, 4:8, :], wp_d[:, 4:8, :])
            nc.scalar.dma_start(pmsk[:], pm_d[:, :])
            nc.gpsimd.dma_start(bg_sb[:], bg_d[:, :])
            nc.gpsimd.dma_start(bu_sb[:], bu_d[:, :])
            for half in range(2):
                nc.gpsimd.dma_start(wd_sb[:, :, ts(half, 512)],
                                    wd_d[:, :, ts(half, 512)])
            nc.gpsimd.dma_start(db_sb[:], db_d[:, :])
            from concourse.masks import make_identity
            make_identity(nc, ident[:])

            emit_at(1)
            for c in range(NTB):
                emit_den(1, c, emit_attn(1, c))
            emit_at(2)
            emit_at(3)
            emit_proj(0)
            emit_proj(1)
            emit_proj(2)
            emit_proj(3)

            # ---- LN2 + transpose to feature-major fp8 -------------------
            for rt in range(RT):
                bst = stats.tile([128, 2, 6], F32, tag="bst2")
                for sg in range(2):
                    nc.vector.bn_stats(bst[:, sg, :], pacc[:, rt, ts(sg, 512)])
                nc.vector.bn_aggr(mv2[rt][:], bst[:])
                std = stats.tile([128, 1], F32, tag="std2")
                nc.scalar.activation(std[:], mv2[rt][:, 1:2],
                                     mybir.ActivationFunctionType.Sqrt,
                                     bias=eps_ap[:])
                rstd2 = stats.tile([128, 1], F32, tag="rstd2")
                nc.vector.reciprocal(rstd2[:], std[:])
                yb = work.tile([128, C], BF16, tag="yb", bufs=2)
                nc.vector.tensor_scalar(yb[:], pacc[:, rt, :], mv2[rt][:, 0:1],
                                        rstd2[:], mybir.AluOpType.subtract,
                                        mybir.AluOpType.mult)
                for half in range(2):
                    pt = psS.tile([128, 4, 128], BF16, tag="s",
                                  name=f"tr2_{rt}_{half}")
                    for i in range(4):
                        cc = half * 4 + i
                        nc.tensor.transpose(pt[:, i, :], yb[:, ts(cc, 128)],
                                            ident[:])
                    nc.vector.tensor_copy(
                        y2T8[:, half * 4:(half + 1) * 4, ts(rt, 128)], pt[:])

        # ---- SwiGLU MLP (row-parallel, fp8 DoubleRow) -------------------
        with ExitStack() as mlp_scope:
            mpool = mlp_scope.enter_context(tc.tile_pool(name="mlp", bufs=1))
            wstream = mlp_scope.enter_context(
                tc.tile_pool(name="wstream", bufs=6))
            psC = mlp_scope.enter_context(
                tc.tile_pool(name="psC", bufs=2, space="PSUM"))

            gu = mpool.tile([128, MCH, ROWS], F8)

            for hc in range(MCH):
                wgt = wstream.tile([128, CCH, 128], F8, tag="wgt")
                nc.sync.dma_start(wgt[:], wg_d[:, hc, :, :])
                wut = wstream.tile([128, CCH, 128], F8, tag="wut")
                nc.sync.dma_start(wut[:], wu_d[:, hc, :, :])
                pg = psC.tile([128, 512], F32, tag="g", bufs=3)
                pu = psC.tile([128, 512], F32, tag="u", bufs=3)
                for c2 in range(CCH // 2):
                    nc.tensor.matmul(pg[:], wgt[:, 2 * c2:2 * c2 + 2, :],
                                     y2T8[:, 2 * c2:2 * c2 + 2, :],
                                     start=(c2 == 0), stop=(c2 == CCH // 2 - 1),
                                     perf_mode=DR)
                for c2 in range(CCH // 2):
                    nc.tensor.matmul(pu[:], wut[:, 2 * c2:2 * c2 + 2, :],
                                     y2T8[:, 2 * c2:2 * c2 + 2, :],
                                     start=(c2 == 0), stop=(c2 == CCH // 2 - 1),
                                     perf_mode=DR)
                gs = work.tile([128, 512], BF16, tag="gs", bufs=2)
                nc.scalar.activation(gs[:], pg[:],
                                     mybir.ActivationFunctionType.Silu,
                                     bias=bg_sb[:, hc:hc + 1])
                us = work.tile([128, 512], BF16, tag="us", bufs=2)
                nc.vector.tensor_scalar_add(us[:], pu[:], bu_sb[:, hc:hc + 1])
                nc.vector.tensor_tensor(gu[:, hc, :], gs[:], us[:],
                                        mybir.AluOpType.mult)

            for tt in range(RT):
                pds = [psC.tile([128, 512], F32, tag="d", name=f"pd_{tt}_{nb}")
                       for nb in range(2)]
                for h2 in range(MCH // 2):
                    for nb in range(2):
                        nc.tensor.matmul(pds[nb][:],
                                         gu[:, 2 * h2:2 * h2 + 2, ts(tt, 128)],
                                         wd_sb[:, 2 * h2:2 * h2 + 2, ts(nb, 512)],
                                         start=(h2 == 0),
                                         stop=(h2 == MCH // 2 - 1),
                                         perf_mode=DR)
                for nb in range(2):
                    o1 = work.tile([128, 512], F32, tag="o1")
                    nc.vector.tensor_tensor(o1[:], pds[nb][:],
                                            pacc[:, tt, ts(nb, 512)],
                                            mybir.AluOpType.add)
                    nc.vector.tensor_tensor(o1[:], o1[:],
                                            db_sb[:, ts(nb, 512)],
                                            mybir.AluOpType.add)
                    nc.sync.dma_start(out_d[ts(tt, 128), ts(nb, 512)], o1[:])

    _split_excess_waits(nc)
    return nc


# ----------------------------------------------------------------------------
# host-side input prep + launch
# ----------------------------------------------------------------------------
_cache = {}

def _get_nc(has_b1):
    if has_b1 not in _cache:
        _cache[has_b1] = _build(has_b1)
    return _cache[has_b1]


def _prep(x, ln1_g, ln1_b, qkv_w, qkv_b, proj_w, proj_b,
          ln2_g, ln2_b, gate_w, gate_b, up_w, up_b, down_w, down_b):
    x = np.asarray(x, np.float32)
    f = lambda a: np.asarray(a, np.float32)
    ln1_g, ln1_b, qkv_b, proj_b, ln2_g, ln2_b = map(f, (
        ln1_g, ln1_b, qkv_b, proj_b, ln2_g, ln2_b))
    qkv_w, proj_w, gate_w, gate_b, up_w, up_b, down_w, down_b = map(f, (
        qkv_w, proj_w, gate_w, gate_b, up_w, up_b, down_w, down_b))

    slopes = _slopes()

    # fold LN affines into the consuming matmuls
    w1 = qkv_w * ln1_g[:, None]
    b1 = ln1_b @ qkv_w + qkv_b              # [3C]
    wg_f = gate_w * ln2_g[:, None]
    bg_f = ln2_b @ gate_w + gate_b          # [HID]
    wu_f = up_w * ln2_g[:, None]
    bu_f = ln2_b @ up_w + up_b

    has_b1 = bool(np.any(b1 != 0.0))

    wgp = np.zeros((C, HIDP), np.float32); wgp[:, :HID] = wg_f
    wup = np.zeros((C, HIDP), np.float32); wup[:, :HID] = wu_f
    wdp = np.zeros((HIDP, 1024), np.float32); wdp[:HID] = down_w
    bgp = np.zeros(HIDP, np.float32); bgp[:HID] = bg_f
    bup = np.zeros(HIDP, np.float32); bup[:HID] = bu_f

    wg_dev = wgp.reshape(CCH, 128, MCH, 128).transpose(1, 2, 0, 3).astype(f8np)
    wu_dev = wup.reshape(CCH, 128, MCH, 128).transpose(1, 2, 0, 3).astype(f8np)
    wd_dev = wdp.reshape(MCH, 128, 1024).transpose(1, 0, 2).astype(f8np)
    bg_dev = bgp.reshape(MCH, 128).T.copy()
    bu_dev = bup.reshape(MCH, 128).T.copy()
    db_dev = np.broadcast_to(down_b, (128, 1024)).copy()

    # triangular diagonal mask (query >= key within a trimmed diag tile)
    pp_i = np.arange(128)[:, None]
    jj = np.arange(512)[None, :]
    tri_np = (jj >= pp_i).astype(bf16)       # [128, 512]

    # full proj weight, chunk 2s+w (w in 0..1): partitions 0:63 = head
    # SLOT_BASE[s]+2w, 64:127 = head SLOT_BASE[s]+2w+1. The AllToAll src
    # chunks u and u+2 share a weight chunk; junk data is masked per core.
    wp_rows = np.empty((CCH, 128, 1024), np.float32)
    for s in range(4):
        for w in range(2):
            for v in range(2):
                hd = SLOT_BASE[s] + 2 * w + v
                wp_rows[2 * s + w, 64 * v:64 * v + 64] = \
                    proj_w[hd * D:(hd + 1) * D, :]
    wp_dev = np.ascontiguousarray(wp_rows.transpose(1, 0, 2)).astype(bf16)

    sel2_np = np.zeros((128, 128), bf16)
    sel2_np[0, 0:64] = 1.0
    sel2_np[64, 64:128] = 1.0

    def qkv_w9(wcols, bcols):
        w9 = np.zeros((2, 2, 128), np.float32)
        w9[0] = wcols.sum(axis=0).reshape(2, 128)
        w9[1] = bcols.reshape(2, 128)
        return w9.astype(bf16)

    in_maps = []
    for core in range(N_CORES):
        b, g = core // TP, core % TP
        heads = [sb + g for sb in SLOT_BASE]        # slot-ordered local heads
        qcols = np.concatenate([np.arange(h * D, (h + 1) * D) for h in heads])
        kcols = qcols + C
        vcols = qcols + 2 * C

        wq_cols = w1[:, qcols] * 0.125           # [C, 256]
        wk_cols = w1[:, kcols]
        wv_cols = w1[:, vcols]
        wq_dev = wq_cols.reshape(CCH, 128, 2, 128).transpose(1, 2, 0, 3).astype(bf16)
        wk_dev = wk_cols.reshape(CCH, 128, 2, 128).transpose(1, 2, 0, 3).astype(bf16)
        wv_dev = wv_cols.reshape(CCH, 128, 256).transpose(1, 0, 2).astype(bf16)
        wq9_dev = qkv_w9(wq_cols, b1[qcols] * 0.125)
        wk9_dev = qkv_w9(wk_cols, b1[kcols])
        wv9_dev = np.zeros((2, 256), np.float32)
        wv9_dev[0] = wv_cols.sum(axis=0)
        wv9_dev[1] = b1[vcols]
        wv9_dev = wv9_dev.astype(bf16)

        # ALiBi key-side factors folded into V (and the denominator column)
        ck = np.zeros((128, NTT, HL), np.float64)
        for hl, h in enumerate(heads):
            sl = slopes[h]
            for t in range(NTT):
                ck[:, t, hl] = np.exp(-sl * (128 * t + np.arange(128)))
        ckf = ck.astype(np.float32)

        xb = x[b]                                # [T, C]
        xT_dev = np.ascontiguousarray(
            xb.T.reshape(CCH, 128, T).transpose(1, 0, 2)).astype(bf16)

        in_maps.append({
            "xT": xT_dev,
            "xtok": xb.astype(bf16),
            "x_rows": xb[512 * g:512 * (g + 1)] + proj_b[None, :],
            "wq": wq_dev, "wk": wk_dev, "wv": wv_dev,
            "wq9": wq9_dev, "wk9": wk9_dev, "wv9": wv9_dev,
            "wproj": wp_dev,
            "pmask": np.broadcast_to(
                np.asarray([1.0 - b, 1.0 - b, float(b), float(b)], np.float32),
                (128, 4)).copy(),
            "wg": wg_dev, "wu": wu_dev, "wd": wd_dev,
            "bg": bg_dev, "bu": bu_dev, "dbias": db_dev,
            "masks": tri_np, "ckb": ckf.astype(bf16), "ckf": ckf,
            "sel2": sel2_np,
        })

    return has_b1, in_maps


def _gather(results):
    out = np.empty((B, T, C), np.float32)
    for core in range(N_CORES):
        b, g = core // TP, core % TP
        out[b, 512 * g:512 * (g + 1)] = results[core]["out_rows"]
    return out


def kernel(**inputs):
    has_b1, in_maps = _prep(**inputs)
    nc = _get_nc(has_b1)
    res = bass_utils.run_bass_kernel_spmd(
        nc, in_maps, core_ids=list(range(N_CORES)))
    return _gather(res.results)


# revision 41
# speedup vs baseline: 1.0105x; 1.0105x over previous
"""Trainium2 Bass kernel for EnhancedTransformerBlock (B=2,T=2048,C=1024,H=16,
SwiGLU HIDDEN=2730, ALiBi-abs + causal attention).

Sharding over 8 cores: batch (2) x head-groups (4 heads/core). Heads are
assigned to cores in decay-sorted slots (slot s on core g = global head
[12,8,4,0][s]+g) so that the per-slot ALiBi key-tile cutoffs [16,15,4,1]
are identical across cores; key tiles whose exp(-slope*k) factor underflows
are skipped structurally (the reference ADDS slope*|q-k|, so weights decay
as exp(-slope*k) after causal masking). LN1 is folded into the QKV matmuls
via a mean-row augmentation plus a per-token rstd scale on the PSUM->SBUF
copy. Attention runs with transposed scores S^T[tk,tq]; the ALiBi key-side
factor is folded into V's rows, the softmax denominator rides as a ck
column through the PV matmul, and causally-dead query columns are trimmed
from diagonal tiles. Attention outputs O^T are exchanged per slot with a
small AllToAll (4x256KB instead of reduce-scattering 4x1MB proj partials);
the full projection then runs on the destination core, accumulating into
an fp32 row accumulator initialized with the residual. The SwiGLU MLP runs
row-parallel with fp8(e4m3) DoubleRow matmuls on contiguous 512-row blocks.
"""
import sys, types
sys.path.insert(0, "/opt/trn_rl_repo")
import numpy as np
import ml_dtypes

import concourse.bass as bass
import concourse.tile as tile
from concourse import mybir
import concourse.bass_utils as bass_utils
import bass_rust

# ----------------------------------------------------------------------------
# environment patches (walrus in this container accepts only 1 sync-wait/inst)
# ----------------------------------------------------------------------------
_DRAIN_WAIT_LIMIT = 1

def _patched_drain_and_barrier(self, tick_clock, wait_clock):
    nc = self.nc
    drain_inst = nc.sync.drain()
    wait_clock.add_sem_waits(
        drain_inst.ins, bass_rust.ScopedClock({None: tick_clock.global_clock})
    )
    si = drain_inst.ins.sync_info
    waits = list(si.on_wait) if si is not None else []
    if len(waits) > _DRAIN_WAIT_LIMIT:
        si.on_wait = waits[:_DRAIN_WAIT_LIMIT]
        for i in range(_DRAIN_WAIT_LIMIT, len(waits), _DRAIN_WAIT_LIMIT):
            d2 = nc.sync.drain()
            d2.ins.sync_info = bass_rust.SyncInfo(
                on_wait=waits[i:i + _DRAIN_WAIT_LIMIT], on_update=[]
            )
    nc.all_engine_barrier()
    popped = nc._tile_sem_poison_stack.pop()
    assert popped is self._sem_poison
    nc.clear_and_free_semaphores(list(self.sems.allocated().values()))
    nc.all_engine_barrier()


def _split_excess_waits(nc, limit=_DRAIN_WAIT_LIMIT):
    n = [0]
    for bb in nc.main_func.blocks:
        insts = bb.instructions
        out = []
        changed = False
        for inst in insts:
            si = inst.sync_info
            waits = list(si.on_wait) if si is not None else []
            if len(waits) > limit:
                changed = True
                keep = waits[-limit:]
                rest = waits[:-limit]
                for i in range(0, len(rest), limit):
                    n[0] += 1
                    d = mybir.InstNoOp(
                        name=f"waitsplit-{n[0]}", engine=inst.engine, ins=[], outs=[]
                    )
                    d.sync_info = bass_rust.SyncInfo(
                        on_wait=rest[i:i + limit], on_update=[]
                    )
                    out.append(d)
                si.on_wait = keep
            out.append(inst)
        if changed:
            bb.instructions = out


def _install_patches():
    tile.TileContext._drain_and_barrier = _patched_drain_and_barrier
    if "antenv.axon_hooks" not in sys.modules:
        try:
            from trn_agent_boot.trn_boot import _ntff_profile_via_ctypes
            hook = _ntff_profile_via_ctypes("/opt/axon/libaxon_pjrt.so")
        except Exception:
            hook = None
        mod = types.ModuleType("antenv.axon_hooks")
        mod.get_axon_ntff_profile_hook = lambda: hook
        mod.set_axon_ntff_profile_hook = lambda h: None
        sys.modules["antenv.axon_hooks"] = mod
        bass_utils.upload_artifacts = lambda tmpdir: tmpdir

_install_patches()

# ----------------------------------------------------------------------------
# problem constants
# ----------------------------------------------------------------------------
B, T, C = 2, 2048, 1024
H, D = 16, 64
HID, HIDP = 2730, 2816
N_CORES, TP = 8, 4
ROWS = T // TP          # 512 contiguous rows per core after the exchange
HL = 4                  # local heads per core (one per slot)
EPS = 1e-5
CBIAS = 24.0
F32, BF16 = mybir.dt.float32, mybir.dt.bfloat16
F8 = mybir.dt.float8e4
bf16 = ml_dtypes.bfloat16
f8np = ml_dtypes.float8_e4m3
ts = bass.ts

NTT = T // 128          # 16 token tiles
NTB = T // 512          # 4 token blocks
CCH = C // 128          # 8 feature chunks
MCH = HIDP // 128       # 22 hidden chunks
RT = ROWS // 128        # 4 row tiles per core
GROUPS = [[0, 1, 2, 3], [4, 5, 6, 7]]
DR = mybir.MatmulPerfMode.DoubleRow

# slot s on core g holds global head SLOT_BASE[s] + g (0-indexed heads).
SLOT_BASE = [12, 8, 4, 0]
# per-slot key-tile cutoff: tile t is kept iff t < TCAP[s]. Computed from
# slope*128*t >= 30 (dropped relative weight < e^{-30+2*Smax} ~ 1e-8).
TCAP = [16, 15, 4, 1]


def _slopes():
    i = np.arange(1, H + 1, dtype=np.float64)
    return 1.0 / np.power(2.0, 8.0 * i / H)


# ----------------------------------------------------------------------------
# device program (identical on all 8 cores; per-core behavior via input data)
# ----------------------------------------------------------------------------
def _build(has_b1):
    K9 = 2 if has_b1 else 1
    nc = bass.Bass("TRN2", num_devices=N_CORES)

    xT_d = nc.dram_tensor("xT", [128, CCH, T], BF16, kind="ExternalInput")
    xtok_d = nc.dram_tensor("xtok", [T, C], BF16, kind="ExternalInput")
    xr_d = nc.dram_tensor("x_rows", [ROWS, C], F32, kind="ExternalInput")
    wq_d = nc.dram_tensor("wq", [128, 2, CCH, 128], BF16, kind="ExternalInput")
    wk_d = nc.dram_tensor("wk", [128, 2, CCH, 128], BF16, kind="ExternalInput")
    wv_d = nc.dram_tensor("wv", [128, CCH, 256], BF16, kind="ExternalInput")
    wq9_d = nc.dram_tensor("wq9", [2, 2, 128], BF16, kind="ExternalInput")
    wk9_d = nc.dram_tensor("wk9", [2, 2, 128], BF16, kind="ExternalInput")
    wv9_d = nc.dram_tensor("wv9", [2, 256], BF16, kind="ExternalInput")
    wp_d = nc.dram_tensor("wproj", [128, CCH, 1024], BF16, kind="ExternalInput")
    pm_d = nc.dram_tensor("pmask", [128, 4], F32, kind="ExternalInput")
    wg_d = nc.dram_tensor("wg", [128, MCH, CCH, 128], F8, kind="ExternalInput")
    wu_d = nc.dram_tensor("wu", [128, MCH, CCH, 128], F8, kind="ExternalInput")
    wd_d = nc.dram_tensor("wd", [128, MCH, 1024], F8, kind="ExternalInput")
    bg_d = nc.dram_tensor("bg", [128, MCH], F32, kind="ExternalInput")
    bu_d = nc.dram_tensor("bu", [128, MCH], F32, kind="ExternalInput")
    db_d = nc.dram_tensor("dbias", [128, 1024], F32, kind="ExternalInput")
    mk_d = nc.dram_tensor("masks", [128, 512], BF16, kind="ExternalInput")
    ckb_d = nc.dram_tensor("ckb", [128, NTT, HL], BF16, kind="ExternalInput")
    ckf_d = nc.dram_tensor("ckf", [128, NTT, HL], F32, kind="ExternalInput")
    sel2_d = nc.dram_tensor("sel2", [128, 128], BF16, kind="ExternalInput")

    out_d = nc.dram_tensor("out_rows", [ROWS, C], F32, kind="ExternalOutput")

    from contextlib import ExitStack
    with tile.TileContext(nc) as tc, ExitStack() as top:
        consts = top.enter_context(tc.tile_pool(name="consts", bufs=1))
        stats = top.enter_context(tc.tile_pool(name="stats", bufs=4))
        work = top.enter_context(tc.tile_pool(name="work", bufs=3))
        dramp = top.enter_context(tc.tile_pool(name="dram", bufs=1, space="DRAM"))

        eps_ap = consts.tile([128, 1], F32)
        nc.vector.memset(eps_ap[:], EPS)
        cb24 = consts.tile([128, 1], F32)
        nc.vector.memset(cb24[:], -CBIAS)
        ones1 = consts.tile([1, 128], F32)
        nc.vector.memset(ones1[:], 1.0)
        ck32 = consts.tile([128, NTT, HL], F32)
        # tiles whose DMAs are deferred past the critical QKV weights
        bg_sb = consts.tile([128, MCH], F32)
        bu_sb = consts.tile([128, MCH], F32)
        ident = consts.tile([128, 128], BF16)
        pmsk = consts.tile([128, 4], F32)
        wd_sb = consts.tile([128, MCH, 1024], F8)
        db_sb = consts.tile([128, 1024], F32)

        # persistent cross-phase state
        pacc = consts.tile([128, RT, C], F32)     # residual + proj accumulator
        y2T8 = consts.tile([128, CCH, ROWS], F8)
        mv2 = [consts.tile([128, 2], F32, name=f"mv2_{r}") for r in range(RT)]

        at_in = [dramp.tile([2 * NTB, 64, 512], F8, name=f"atin{s}")
                 for s in range(4)]
        dz_in = dramp.tile([8, 128], F8, name="dzin")
        dz_out = dramp.tile([8, 128], F8, name="dzout")
        at_out = [dramp.tile([2 * NTB, 64, 512], F8, name=f"atout{s}")
                  for s in range(4)]

        with ExitStack() as attn_scope:
            apool = attn_scope.enter_context(tc.tile_pool(name="attn", bufs=1))
            pipe = attn_scope.enter_context(tc.tile_pool(name="pipe", bufs=4))
            OnTp = attn_scope.enter_context(tc.tile_pool(name="OnTp", bufs=3))
            recp = attn_scope.enter_context(tc.tile_pool(name="recp", bufs=2))
            psA = attn_scope.enter_context(
                tc.tile_pool(name="psA", bufs=2, space="PSUM"))
            psS = attn_scope.enter_context(
                tc.tile_pool(name="psS", bufs=3, space="PSUM"))
            psO = attn_scope.enter_context(
                tc.tile_pool(name="psO", bufs=3, space="PSUM"))

            # qT/kT: [d 0:64 = slot 2p head, 64:128 = slot 2p+1 head][pair][T]
            qT = apool.tile([128, 2, T], BF16)
            kT = apool.tile([128, 2, T], BF16)
            # critical-path weight DMAs first on the gpsimd queue
            zt8 = apool.tile([1, 8, 128], F8)
            nc.vector.memset(zt8[:], 0.0)
            nc.sync.dma_start(dz_in[:, :], zt8[0, :, :])
            nc.gpsimd.collective_compute(
                "AllToAll", mybir.AluOpType.bypass,
                replica_groups=[[0, 1, 2, 3, 4, 5, 6, 7]],
                ins=[dz_in.opt()], outs=[dz_out.opt()],
            )
            wq_sb = apool.tile([128, 2, CCH, 128], BF16)
            nc.gpsimd.dma_start(wq_sb[:], wq_d[:, :, :, :])
            wk_sb = apool.tile([128, 2, CCH, 128], BF16)
            nc.gpsimd.dma_start(wk_sb[:], wk_d[:, :, :, :])
            wv_sb = apool.tile([128, CCH, 256], BF16)
            nc.gpsimd.dma_start(wv_sb[:], wv_d[:, :, :])
            wq9_sb = apool.tile([2, 2, 128], BF16)
            nc.gpsimd.dma_start(wq9_sb[:], wq9_d[:, :, :])
            wk9_sb = apool.tile([2, 2, 128], BF16)
            nc.gpsimd.dma_start(wk9_sb[:], wk9_d[:, :, :])
            wv9_sb = apool.tile([2, 256], BF16)
            nc.gpsimd.dma_start(wv9_sb[:], wv9_d[:, :])
            tri = apool.tile([128, 512], BF16)
            nc.gpsimd.dma_start(tri[:], mk_d[:, :])
            nc.gpsimd.dma_start(ck32[:], ckf_d[:, :, :])
            sel2 = apool.tile([128, 128], BF16)
            nc.gpsimd.dma_start(sel2[:], sel2_d[:, :])
            dpad2s = []
            for i in range(2):
                dp = apool.tile([128, 512], BF16, name=f"dpad{i}")
                nc.vector.memset(dp[:], 0.0)
                dpad2s.append(dp)
            Vh = apool.tile([128, NTT, HL, 66], BF16)
            nc.vector.memset(Vh[:, :, :, 65:66], 0.0)
            nc.gpsimd.dma_start(Vh[:, :, :, 64:65], ckb_d[:, :, :])
            wp_sb = apool.tile([128, CCH, 1024], BF16)
            for rt in range(RT):
                nc.scalar.dma_start(pacc[:, rt, :], xr_d[ts(rt, 128), :])

            with ExitStack() as qkv_scope:
                qpool = qkv_scope.enter_context(tc.tile_pool(name="qkv", bufs=1))

                xT_sb = qpool.tile([128, CCH, T], BF16)
                for cc in range(CCH):
                    nc.sync.dma_start(xT_sb[:, cc, :], xT_d[:, cc, :])

                mrow = qpool.tile([2, T], BF16)    # row0: -mu, row1: ones(b1)
                nc.vector.memset(mrow[:], 0.0)
                if has_b1:
                    nc.vector.memset(mrow[1:2, :], 1.0)
                rstd_row = qpool.tile([1, T], F32)
                rstdb = [qpool.tile([128, 512], F32, name=f"rstdb{b}")
                         for b in range(NTB)]
                rstds = [qpool.tile([128, 1], F32, name=f"rstds{t}")
                         for t in range(NTT)]

                def emit_stats(b):
                    # per-token-tile LN stats for block b (vector engine)
                    mucB = stats.tile([128, 32], F32, tag="mucB", bufs=2)
                    nc.vector.memset(mucB[:], 0.0)
                    for i in range(4):
                        t = 4 * b + i
                        xt = work.tile([128, C], BF16, tag="xt", bufs=2)
                        nc.scalar.dma_start(xt[:], xtok_d[ts(t, 128), :])
                        bst = stats.tile([128, 2, 6], F32, tag="bst")
                        for sg in range(2):
                            nc.vector.bn_stats(bst[:, sg, :], xt[:, ts(sg, 512)])
                        mv = stats.tile([128, 2], F32, tag="mv")
                        nc.vector.bn_aggr(mv[:], bst[:])
                        std = stats.tile([128, 1], F32, tag="std")
                        nc.scalar.activation(std[:], mv[:, 1:2],
                                             mybir.ActivationFunctionType.Sqrt,
                                             bias=eps_ap[:])
                        nc.vector.reciprocal(rstds[t][:], std[:])
                        nc.vector.tensor_scalar_mul(mucB[:, i:i + 1],
                                                    mv[:, 0:1], -1.0)
                        nc.vector.tensor_copy(mucB[:, 4 + i:5 + i], rstds[t][:])
                    stt = stats.tile([128, 32], F32, tag="stt")
                    nc.vector.transpose(stt[:], mucB[:])
                    sTb = stats.tile([8, 128], BF16, tag="sTb")
                    sTf = stats.tile([8, 128], F32, tag="sTf")
                    for i in range(4):
                        nc.vector.tensor_copy(sTb[0:8, ts(i, 32)],
                                              stt[32 * i:32 * i + 8, :])
                        nc.vector.tensor_copy(sTf[0:8, ts(i, 32)],
                                              stt[32 * i:32 * i + 8, :])
                    for i in range(4):
                        nc.scalar.dma_start(
                            mrow[0:1, 512 * b + 128 * i:512 * b + 128 * (i + 1)],
                            sTb[i:i + 1, :])
                        nc.scalar.dma_start(
                            rstd_row[0:1, 512 * b + 128 * i:512 * b + 128 * (i + 1)],
                            sTf[4 + i:5 + i, :])

                def emit_qkv(b):
                    def qk_cc(p, wi, wsb):
                        ps = psA.tile([128, 512], F32, tag="ps",
                                      name=f"qk_{b}_{p}_{wi}")
                        for cc in range(CCH):
                            nc.tensor.matmul(
                                ps[:], wsb[:, p, cc, :],
                                xT_sb[:, cc, ts(b, 512)],
                                start=(cc == 0), stop=False)
                        return ps

                    def qk_mean(ps, p, w9):
                        nc.tensor.matmul(
                            ps[:], w9[0:K9, p, :],
                            mrow[0:K9, ts(b, 512)],
                            start=False, stop=True)

                    def psv_cc(t):
                        psv = psS.tile([128, 256], F32, tag="s",
                                       name=f"v_{t}")
                        for cc in range(CCH):
                            nc.tensor.matmul(psv[:], xT_sb[:, cc, ts(t, 128)],
                                             wv_sb[:, cc, :],
                                             start=(cc == 0), stop=False)
                        return psv

                    def psv_mean(t, psv):
                        nc.tensor.matmul(psv[:], mrow[0:K9, ts(t, 128)],
                                         wv9_sb[0:K9, :], start=False,
                                         stop=True)
                        for h in range(HL):
                            nc.vector.tensor_scalar(
                                Vh[:, t, h, 0:64], psv[:, ts(h, 64)],
                                rstds[t][:], ck32[:, t, h:h + 1],
                                mybir.AluOpType.mult, mybir.AluOpType.mult)

                    for half in range(2):
                        ps_q = qk_cc(half, 0, wq_sb)
                        ps_k = qk_cc(half, 1, wk_sb)
                        t0, t1 = 4 * b + 2 * half, 4 * b + 2 * half + 1
                        psv0 = psv_cc(t0)
                        psv1 = psv_cc(t1)
                        # stats-dependent matmuls after a long independent run
                        qk_mean(ps_q, half, wq9_sb)
                        qk_mean(ps_k, half, wk9_sb)
                        if half == 0:
                            psb = psS.tile([128, 512], F32, tag="s",
                                           name=f"psb_{b}")
                            nc.tensor.matmul(psb[:], ones1[0:1, :],
                                             rstd_row[0:1, ts(b, 512)],
                                             start=True, stop=True)
                            nc.scalar.copy(rstdb[b][:], psb[:])
                        nc.vector.tensor_tensor(qT[:, half, ts(b, 512)],
                                                ps_q[:], rstdb[b][:],
                                                mybir.AluOpType.mult)
                        nc.vector.tensor_tensor(kT[:, half, ts(b, 512)],
                                                ps_k[:], rstdb[b][:],
                                                mybir.AluOpType.mult)
                        psv_mean(t0, psv0)
                        psv_mean(t1, psv1)

                # ---- attention for pair p, query block c -----------------
                npair = [0]

                def emit_attn(p, c):
                    caps = (min(TCAP[2 * p], 4 * c + 4),
                            min(TCAP[2 * p + 1], 4 * c + 4))
                    t_hi = max(caps)
                    pos = [psO.tile([66, 512], F32, tag="po",
                                    name=f"po{j}_{c}_{p}") for j in range(2)]

                    def emit_s(t):
                        # S^T tiles for both heads of the pair at key tile t
                        if t >= 4 * c:
                            w = 512 - 128 * (t - 4 * c)
                            diag = True
                        else:
                            w, diag = 512, False
                        q0 = 512 * c + 512 - w
                        outs = {}
                        for j in range(2):
                            if t >= caps[j]:
                                continue
                            r0 = 64 * j
                            st = psS.tile([128, 512], F32, tag="s",
                                          name=f"s_{c}_{p}_{j}_{t}")
                            nc.tensor.matmul(st[:, 0:w],
                                             kT[r0:r0 + 64, p, ts(t, 128)],
                                             qT[r0:r0 + 64, p, q0:q0 + w],
                                             start=True, stop=True)
                            pT = pipe.tile([128, 512], BF16, tag="pT",
                                           name=f"pT_{c}_{p}_{j}_{t}")
                            nc.scalar.activation(
                                pT[:, 0:w], st[:, 0:w],
                                mybir.ActivationFunctionType.Exp,
                                bias=cb24[:])
                            if diag:
                                nc.vector.tensor_tensor(
                                    pT[:, 0:w], pT[:, 0:w],
                                    tri[:, 0:w], mybir.AluOpType.mult)
                            outs[j] = (pT, w)
                        return outs

                    pTs = {0: emit_s(0)}
                    if t_hi > 1:
                        pTs[1] = emit_s(1)
                    for t in range(t_hi):
                        outs = pTs.pop(t)
                        for j in range(2):
                            if j not in outs:
                                continue
                            pT, w = outs[j]
                            nc.tensor.matmul(pos[j][:, 512 - w:512],
                                             Vh[:, t, 2 * p + j, 0:66],
                                             pT[:, 0:w], start=(t == 0),
                                             stop=(t == caps[j] - 1),
                                             skip_group_check=True)
                        if t + 2 < t_hi:
                            pTs[t + 2] = emit_s(t + 2)

                    return pos

                def emit_den(p, c, pos):
                    # softmax denominators for the pair -> broadcast
                    dp = dpad2s[npair[0] % 2]
                    npair[0] += 1
                    nc.vector.tensor_copy(dp[0:1, :], pos[0][64:65, :])
                    nc.vector.tensor_copy(dp[64:65, :], pos[1][64:65, :])
                    rb = psS.tile([128, 512], F32, tag="s", name=f"rb_{c}_{p}")
                    nc.tensor.matmul(rb[:], sel2[:], dp[:],
                                     start=True, stop=True)
                    rec = recp.tile([128, 512], F32, tag="rec")
                    nc.vector.reciprocal(rec[:], rb[:])
                    OnT_c = OnTp.tile([128, 512], F8, tag="OnT")
                    nc.vector.tensor_tensor(OnT_c[0:64, :],
                                            pos[0][0:64, :], rec[0:64, :],
                                            mybir.AluOpType.mult)
                    nc.vector.tensor_tensor(OnT_c[64:128, :],
                                            pos[1][0:64, :],
                                            rec[64:128, :],
                                            mybir.AluOpType.mult)
                    for j in range(2):
                        # write to both batches' destination slots (the
                        # other batch's copy is zero-masked before proj)
                        nc.sync.dma_start(at_in[2 * p + j][c, :, :],
                                          OnT_c[64 * j:64 * j + 64, :])
                        nc.sync.dma_start(at_in[2 * p + j][4 + c, :, :],
                                          OnT_c[64 * j:64 * j + 64, :])

                # ---- emission: stats ahead of QKV, slot 0 after ----------
                emit_stats(0)
                emit_stats(1)
                emit_stats(2)
                emit_stats(3)
                emit_qkv(0)
                emit_qkv(1)
                emit_qkv(2)
                emit_qkv(3)
                for c in range(NTB):
                    emit_den(0, c, emit_attn(0, c))

            ALL8 = [[0, 1, 2, 3, 4, 5, 6, 7]]

            def emit_at(s):
                nc.gpsimd.collective_compute(
                    "AllToAll", mybir.AluOpType.bypass,
                    replica_groups=ALL8,
                    ins=[at_in[s].opt()], outs=[at_out[s].opt()],
                )

            def emit_proj(s):
                ot8 = OnTp.tile([128, 4, 512], F8, tag="ot8", bufs=2,
                                name=f"ot8_{s}")
                ot = OnTp.tile([128, 4, 512], BF16, tag="ot", bufs=2,
                               name=f"ot_{s}")
                for u in range(4):
                    nc.sync.dma_start(ot8[:, u, :],
                                      at_out[s][2 * u:2 * u + 2, :, :])
                    # upcast to bf16, zeroing the other batch's junk chunk
                    nc.vector.tensor_scalar_mul(ot[:, u, :], ot8[:, u, :],
                                                pmsk[:, u:u + 1])
                for rt in range(RT):
                    for nb in range(2):
                        pp = psA.tile([128, 512], F32, tag="ps",
                                      name=f"pj_{s}_{rt}_{nb}")
                        for u in range(4):
                            nc.tensor.matmul(
                                pp[:], ot[:, u, ts(rt, 128)],
                                wp_sb[:, 2 * s + (u % 2), ts(nb, 512)],
                                start=(u == 0), stop=(u == 3))
                        nc.vector.tensor_tensor(pacc[:, rt, ts(nb, 512)],
                                                pacc[:, rt, ts(nb, 512)],
                                                pp[:], mybir.AluOpType.add)

            emit_at(0)
            # deferred non-critical DMAs: dispatched once slot-0 work is off
            # the queues, landing during the attention phase
            nc.scalar.dma_start(wp_sb[:, 0:4, :], wp_d[:, 0:4, :])
            nc.scalar.dma_start(wp_sb[:, 4:8, :], wp_d[:, 4:8, :])
            nc.scalar.dma_start(pmsk[:], pm_d[:, :])
            nc.gpsimd.dma_start(bg_sb[:], bg_d[:, :])
            nc.gpsimd.dma_start(bu_sb[:], bu_d[:, :])
            for half in range(2):
                nc.gpsimd.dma_start(wd_sb[:, :, ts(half, 512)],
                                    wd_d[:, :, ts(half, 512)])
            nc.gpsimd.dma_start(db_sb[:], db_d[:, :])
            from concourse.masks import make_identity
            make_identity(nc, ident[:])

            emit_at(1)
            for c in range(NTB):
                emit_den(1, c, emit_attn(1, c))
            emit_at(2)
            emit_at(3)
            emit_proj(0)
            emit_proj(1)
            emit_proj(2)
            emit_proj(3)

            # ---- LN2 + transpose to feature-major fp8 -------------------
            for rt in range(RT):
                bst = stats.tile([128, 2, 6], F32, tag="bst2")
                for sg in range(2):
                    nc.vector.bn_stats(bst[:, sg, :], pacc[:, rt, ts(sg, 512)])
                nc.vector.bn_aggr(mv2[rt][:], bst[:])
                std = stats.tile([128, 1], F32, tag="std2")
                nc.scalar.activation(std[:], mv2[rt][:, 1:2],
                                     mybir.ActivationFunctionType.Sqrt,
                                     bias=eps_ap[:])
                rstd2 = stats.tile([128, 1], F32, tag="rstd2")
                nc.vector.reciprocal(rstd2[:], std[:])
                yb = work.tile([128, C], BF16, tag="yb", bufs=2)
                nc.vector.tensor_scalar(yb[:], pacc[:, rt, :], mv2[rt][:, 0:1],
                                        rstd2[:], mybir.AluOpType.subtract,
                                        mybir.AluOpType.mult)
                for half in range(2):
                    pt = psS.tile([128, 4, 128], BF16, tag="s",
                                  name=f"tr2_{rt}_{half}")
                    for i in range(4):
                        cc = half * 4 + i
                        nc.tensor.transpose(pt[:, i, :], yb[:, ts(cc, 128)],
                                            ident[:])
                    nc.vector.tensor_copy(
                        y2T8[:, half * 4:(half + 1) * 4, ts(rt, 128)], pt[:])

        # ---- SwiGLU MLP (row-parallel, fp8 DoubleRow) -------------------
        with ExitStack() as mlp_scope:
            mpool = mlp_scope.enter_context(tc.tile_pool(name="mlp", bufs=1))
            wstream = mlp_scope.enter_context(
                tc.tile_pool(name="wstream", bufs=6))
            psC = mlp_scope.enter_context(
                tc.tile_pool(name="psC", bufs=2, space="PSUM"))

            gu = mpool.tile([128, MCH, ROWS], F8)

            for hc in range(MCH):
                wgt = wstream.tile([128, CCH, 128], F8, tag="wgt")
                nc.sync.dma_start(wgt[:], wg_d[:, hc, :, :])
                wut = wstream.tile([128, CCH, 128], F8, tag="wut")
                nc.sync.dma_start(wut[:], wu_d[:, hc, :, :])
                pg = psC.tile([128, 512], F32, tag="g", bufs=3)
                pu = psC.tile([128, 512], F32, tag="u", bufs=3)
                for c2 in range(CCH // 2):
                    nc.tensor.matmul(pg[:], wgt[:, 2 * c2:2 * c2 + 2, :],
                                     y2T8[:, 2 * c2:2 * c2 + 2, :],
                                     start=(c2 == 0), stop=(c2 == CCH // 2 - 1),
                                     perf_mode=DR)
                for c2 in range(CCH // 2):
                    nc.tensor.matmul(pu[:], wut[:, 2 * c2:2 * c2 + 2, :],
                                     y2T8[:, 2 * c2:2 * c2 + 2, :],
                                     start=(c2 == 0), stop=(c2 == CCH // 2 - 1),
                                     perf_mode=DR)
                gs = work.tile([128, 512], BF16, tag="gs", bufs=2)
                nc.scalar.activation(gs[:], pg[:],
                                     mybir.ActivationFunctionType.Silu,
                                     bias=bg_sb[:, hc:hc + 1])
                us = work.tile([128, 512], BF16, tag="us", bufs=2)
                nc.vector.tensor_scalar_add(us[:], pu[:], bu_sb[:, hc:hc + 1])
                nc.vector.tensor_tensor(gu[:, hc, :], gs[:], us[:],
                                        mybir.AluOpType.mult)

            for tt in range(RT):
                pds = [psC.tile([128, 512], F32, tag="d", name=f"pd_{tt}_{nb}")
                       for nb in range(2)]
                for h2 in range(MCH // 2):
                    for nb in range(2):
                        nc.tensor.matmul(pds[nb][:],
                                         gu[:, 2 * h2:2 * h2 + 2, ts(tt, 128)],
                                         wd_sb[:, 2 * h2:2 * h2 + 2, ts(nb, 512)],
                                         start=(h2 == 0),
                                         stop=(h2 == MCH // 2 - 1),
                                         perf_mode=DR)
                for nb in range(2):
                    o1 = work.tile([128, 512], F32, tag="o1")
                    nc.vector.tensor_tensor(o1[:], pds[nb][:],
                                            pacc[:, tt, ts(nb, 512)],
                                            mybir.AluOpType.add)
                    nc.vector.tensor_tensor(o1[:], o1[:],
                                            db_sb[:, ts(nb, 512)],
                                            mybir.AluOpType.add)
                    nc.sync.dma_start(out_d[ts(tt, 128), ts(nb, 512)], o1[:])

    _split_excess_waits(nc)
    return nc


# ----------------------------------------------------------------------------
# host-side input prep + launch
# ----------------------------------------------------------------------------
_cache = {}

def _get_nc(has_b1):
    if has_b1 not in _cache:
        _cache[has_b1] = _build(has_b1)
    return _cache[has_b1]


def _prep(x, ln1_g, ln1_b, qkv_w, qkv_b, proj_w, proj_b,
          ln2_g, ln2_b, gate_w, gate_b, up_w, up_b, down_w, down_b):
    x = np.asarray(x, np.float32)
    f = lambda a: np.asarray(a, np.float32)
    ln1_g, ln1_b, qkv_b, proj_b, ln2_g, ln2_b = map(f, (
        ln1_g, ln1_b, qkv_b, proj_b, ln2_g, ln2_b))
    qkv_w, proj_w, gate_w, gate_b, up_w, up_b, down_w, down_b = map(f, (
        qkv_w, proj_w, gate_w, gate_b, up_w, up_b, down_w, down_b))

    slopes = _slopes()

    # fold LN affines into the consuming matmuls
    w1 = qkv_w * ln1_g[:, None]
    b1 = ln1_b @ qkv_w + qkv_b              # [3C]
    wg_f = gate_w * ln2_g[:, None]
    bg_f = ln2_b @ gate_w + gate_b          # [HID]
    wu_f = up_w * ln2_g[:, None]
    bu_f = ln2_b @ up_w + up_b

    has_b1 = bool(np.any(b1 != 0.0))

    wgp = np.zeros((C, HIDP), np.float32); wgp[:, :HID] = wg_f
    wup = np.zeros((C, HIDP), np.float32); wup[:, :HID] = wu_f
    wdp = np.zeros((HIDP, 1024), np.float32); wdp[:HID] = down_w
    bgp = np.zeros(HIDP, np.float32); bgp[:HID] = bg_f
    bup = np.zeros(HIDP, np.float32); bup[:HID] = bu_f

    wg_dev = wgp.reshape(CCH, 128, MCH, 128).transpose(1, 2, 0, 3).astype(f8np)
    wu_dev = wup.reshape(CCH, 128, MCH, 128).transpose(1, 2, 0, 3).astype(f8np)
    wd_dev = wdp.reshape(MCH, 128, 1024).transpose(1, 0, 2).astype(f8np)
    bg_dev = bgp.reshape(MCH, 128).T.copy()
    bu_dev = bup.reshape(MCH, 128).T.copy()
    db_dev = np.broadcast_to(down_b, (128, 1024)).copy()

    # triangular diagonal mask (query >= key within a trimmed diag tile)
    pp_i = np.arange(128)[:, None]
    jj = np.arange(512)[None, :]
    tri_np = (jj >= pp_i).astype(bf16)       # [128, 512]

    # full proj weight, chunk 2s+w (w in 0..1): partitions 0:63 = head
    # SLOT_BASE[s]+2w, 64:127 = head SLOT_BASE[s]+2w+1. The AllToAll src
    # chunks u and u+2 share a weight chunk; junk data is masked per core.
    wp_rows = np.empty((CCH, 128, 1024), np.float32)
    for s in range(4):
        for w in range(2):
            for v in range(2):
                hd = SLOT_BASE[s] + 2 * w + v
                wp_rows[2 * s + w, 64 * v:64 * v + 64] = \
                    proj_w[hd * D:(hd + 1) * D, :]
    wp_dev = np.ascontiguousarray(wp_rows.transpose(1, 0, 2)).astype(bf16)

    sel2_np = np.zeros((128, 128), bf16)
    sel2_np[0, 0:64] = 1.0
    sel2_np[64, 64:128] = 1.0

    def qkv_w9(wcols, bcols):
        w9 = np.zeros((2, 2, 128), np.float32)
        w9[0] = wcols.sum(axis=0).reshape(2, 128)
        w9[1] = bcols.reshape(2, 128)
        return w9.astype(bf16)

    in_maps = []
    for core in range(N_CORES):
        b, g = core // TP, core % TP
        heads = [sb + g for sb in SLOT_BASE]        # slot-ordered local heads
        qcols = np.concatenate([np.arange(h * D, (h + 1) * D) for h in heads])
        kcols = qcols + C
        vcols = qcols + 2 * C

        wq_cols = w1[:, qcols] * 0.125           # [C, 256]
        wk_cols = w1[:, kcols]
        wv_cols = w1[:, vcols]
        wq_dev = wq_cols.reshape(CCH, 128, 2, 128).transpose(1, 2, 0, 3).astype(bf16)
        wk_dev = wk_cols.reshape(CCH, 128, 2, 128).transpose(1, 2, 0, 3).astype(bf16)
        wv_dev = wv_cols.reshape(CCH, 128, 256).transpose(1, 0, 2).astype(bf16)
        wq9_dev = qkv_w9(wq_cols, b1[qcols] * 0.125)
        wk9_dev = qkv_w9(wk_cols, b1[kcols])
        wv9_dev = np.zeros((2, 256), np.float32)
        wv9_dev[0] = wv_cols.sum(axis=0)
        wv9_dev[1] = b1[vcols]
        wv9_dev = wv9_dev.astype(bf16)

        # ALiBi key-side factors folded into V (and the denominator column)
        ck = np.zeros((128, NTT, HL), np.float64)
        for hl, h in enumerate(heads):
            sl = slopes[h]
            for t in range(NTT):
                ck[:, t, hl] = np.exp(-sl * (128 * t + np.arange(128)))
        ckf = ck.astype(np.float32)

        xb = x[b]                                # [T, C]
        xT_dev = np.ascontiguousarray(
            xb.T.reshape(CCH, 128, T).transpose(1, 0, 2)).astype(bf16)

        in_maps.append({
            "xT": xT_dev,
            "xtok": xb.astype(bf16),
            "x_rows": xb[512 * g:512 * (g + 1)] + proj_b[None, :],
            "wq": wq_dev, "wk": wk_dev, "wv": wv_dev,
            "wq9": wq9_dev, "wk9": wk9_dev, "wv9": wv9_dev,
            "wproj": wp_dev,
            "pmask": np.broadcast_to(
                np.asarray([1.0 - b, 1.0 - b, float(b), float(b)], np.float32),
                (128, 4)).copy(),
            "wg": wg_dev, "wu": wu_dev, "wd": wd_dev,
            "bg": bg_dev, "bu": bu_dev, "dbias": db_dev,
            "masks": tri_np, "ckb": ckf.astype(bf16), "ckf": ckf,
            "sel2": sel2_np,
        })

    return has_b1, in_maps


def _gather(results):
    out = np.empty((B, T, C), np.float32)
    for core in range(N_CORES):
        b, g = core // TP, core % TP
        out[b, 512 * g:512 * (g + 1)] = results[core]["out_rows"]
    return out


def kernel(**inputs):
    has_b1, in_maps = _prep(**inputs)
    nc = _get_nc(has_b1)
    res = bass_utils.run_bass_kernel_spmd(
        nc, in_maps, core_ids=list(range(N_CORES)))
    return _gather(res.results)
